# revision 11
# baseline (speedup 1.0000x reference)
"""Trainium2 Bass kernel for nn_LlamaAttention (T=2048, HID=4096, HQ=32, HKV=8, D=128).

Tensor-parallel over heads across 8 NeuronCores: core c owns q-heads 4c..4c+3 and
kv-head c (GQA group size 4 == heads-per-core, so attention is fully core-local).
Wo is row-sharded; each core computes a partial [T, HID] output (transposed) and
the host sums the 8 partials. No device collectives.

Self-contained: hardcodes all shapes; builds the Bass kernel once per process.
"""
import numpy as np

T, HID, HQ, HKV, D = 2048, 4096, 32, 8, 128
NCORES = 8
HPC = HQ // NCORES            # 4 q heads per core
QW = HPC * D                  # 512 q columns per core
KO = HID // 128               # 32 k-tiles
ACH = 256                     # phase A T-chunk width (f32r needs N>=256)
NACH = T // ACH               # 8
CH = 512                      # attention tq chunk width
NCH = T // CH                 # 4
SCALING = float(D) ** -0.5

_CACHE = {}


def _build_nc():
    import concourse.mybir as mybir
    import concourse.tile as tile
    from concourse import bacc
    from contextlib import ExitStack

    F32 = mybir.dt.float32
    F32R = mybir.dt.float32r
    AF = mybir.ActivationFunctionType

    nc = bacc.Bacc("TRN2", target_bir_lowering=False, debug=False,
                   dynamic_dma_scratch_size=2048)

    hidT = nc.dram_tensor("hidT", [HID, T], F32R, kind="ExternalInput")
    wqkv = nc.dram_tensor("wqkv", [HID, QW + 2 * D], F32R, kind="ExternalInput")
    wo = nc.dram_tensor("wo", [QW, HID], F32R, kind="ExternalInput")
    cosT = nc.dram_tensor("cosT", [D, T], F32, kind="ExternalInput")
    sinT = nc.dram_tensor("sinT", [D, T], F32, kind="ExternalInput")
    rmatT = nc.dram_tensor("rmatT", [D, D], F32R, kind="ExternalInput")
    onesd = nc.dram_tensor("onesd", [128, 128], F32R, kind="ExternalInput")
    identd = nc.dram_tensor("identd", [128, 128], F32R, kind="ExternalInput")
    masks = nc.dram_tensor("masks", [4, 128, CH], F32, kind="ExternalInput")
    outT = nc.dram_tensor("outT_p", [HID, T], F32, kind="ExternalOutput")

    with tile.TileContext(nc) as tc, ExitStack() as ctx:
        consts = ctx.enter_context(tc.tile_pool(name="consts", bufs=1))
        ones_sb = consts.tile([128, 128], F32R)
        nc.sync.dma_start(ones_sb[:], onesd[:, :])
        ident = consts.tile([128, 128], F32R)
        nc.sync.dma_start(ident[:], identd[:, :])
        rmatT_sb = consts.tile([128, 128], F32R)
        nc.sync.dma_start(rmatT_sb[:], rmatT[:, :])

        qkv_pool = ctx.enter_context(tc.tile_pool(name="qkvT", bufs=1))
        # 5 blocks of qkvT: m=0..3 q heads, m=4 kT (v goes via staging tiles)
        qk_t = [qkv_pool.tile([128, T], F32R, name=f"qkvT{m}") for m in range(5)]
        v_sb = qkv_pool.tile([128, T // 128, 128], F32R, name="v_sb")  # [ts, blk, D]

        # ---------------- Phase A: qkvT = wqkv.T @ hidT, rope fused ----------
        with tc.tile_pool(name="wqkv", bufs=1) as wp, \
             tc.tile_pool(name="hid", bufs=2) as hp, \
             tc.tile_pool(name="vstage", bufs=2) as vsp, \
             tc.tile_pool(name="cs", bufs=4) as csp, \
             tc.tile_pool(name="ropetmp", bufs=2) as tmp_pool, \
             tc.tile_pool(name="psA", bufs=3, space="PSUM") as psA, \
             tc.tile_pool(name="psT", bufs=2, space="PSUM") as psT, \
             tc.tile_pool(name="psR", bufs=2, space="PSUM") as psR:
            hid3 = hidT.rearrange("(ko p) t -> p ko t", p=128)
            w3 = wqkv.rearrange("(ko p) m -> p ko m", p=128)
            # first hid chunk before weights so compute starts ASAP
            h_tiles = {}
            h_tiles[0] = hp.tile([128, KO, ACH], F32R, tag="hid", name="hid0")
            nc.sync.dma_start(h_tiles[0][:], hid3[:, :, 0:ACH])
            w_sb = []
            for m in range(6):
                wt = wp.tile([128, KO, 128], F32R, name=f"w{m}")
                nc.sync.dma_start(wt[:], w3[:, :, m * 128:(m + 1) * 128])
                w_sb.append(wt)
            for ch in range(NACH):
                asl = slice(ch * ACH, (ch + 1) * ACH)
                if ch not in h_tiles:
                    h_tiles[ch] = hp.tile([128, KO, ACH], F32R, tag="hid", name=f"hid{ch}")
                    nc.sync.dma_start(h_tiles[ch][:], hid3[:, :, asl])
                h_sb = h_tiles[ch]
                cos_ch = csp.tile([128, ACH], F32, tag="cos")
                nc.sync.dma_start(cos_ch[:], cosT[:, asl])
                sin_ch = csp.tile([128, ACH], F32, tag="sin")
                nc.sync.dma_start(sin_ch[:], sinT[:, asl])
                for m in range(6):
                    ps = psA.tile([128, ACH], F32, tag="psA")
                    for k in range(KO):
                        nc.tensor.matmul(
                            ps[:], w_sb[m][:, k, :], h_sb[:, k, :],
                            start=(k == 0), stop=(k == KO - 1))
                    if m < 5:
                        nc.vector.tensor_copy(qk_t[m][:, asl], ps[:])
                        # fused RoPE on this chunk: x = x*cos + (rmatT.T@x)*sin
                        rp = psR.tile([128, ACH], F32, tag="psR")
                        nc.tensor.matmul(rp[:], rmatT_sb[:], qk_t[m][:, asl],
                                         start=True, stop=True)
                        tmp = tmp_pool.tile([128, ACH], F32, tag="tmp")
                        nc.vector.tensor_mul(tmp[:], rp[:], sin_ch[:])
                        nc.vector.tensor_mul(qk_t[m][:, asl], qk_t[m][:, asl], cos_ch[:])
                        nc.vector.tensor_add(qk_t[m][:, asl], qk_t[m][:, asl], tmp[:])
                    else:
                        vstage = vsp.tile([128, ACH], F32R, tag="vst")
                        nc.vector.tensor_copy(vstage[:], ps[:])
                        for jj in range(ACH // 128):
                            j = ch * (ACH // 128) + jj
                            pst = psT.tile([128, 128], F32R, tag="psT")
                            nc.tensor.transpose(
                                pst[:], vstage[:, jj * 128:(jj + 1) * 128], ident[:])
                            nc.vector.tensor_copy(v_sb[:, j, :], pst[:])

        # ---------------- Phase B consts ----------------
        bconsts = ctx.enter_context(tc.tile_pool(name="bconsts", bufs=1))
        mask_sb = bconsts.tile([128, 4, CH], F32)
        nc.sync.dma_start(mask_sb[:], masks.rearrange("i p f -> p i f"))

        wop = ctx.enter_context(tc.tile_pool(name="wo", bufs=1))
        wo_sb = wop.tile([128, HPC, HID // 128, 128], F32R)
        nc.sync.dma_start(
            wo_sb[:], wo.rearrange("(kk p) (mo q) -> p kk mo q", p=128, q=128))

        probs_pool = ctx.enter_context(tc.tile_pool(name="probs", bufs=5))
        attn_pool = ctx.enter_context(tc.tile_pool(name="attnT", bufs=1))
        attn_sb = [attn_pool.tile([128, T], F32R, name=f"attnT{h}") for h in range(HPC)]
        rden_pool = ctx.enter_context(tc.tile_pool(name="rden", bufs=2))
        out_pool = ctx.enter_context(tc.tile_pool(name="outstage", bufs=4))
        psS = ctx.enter_context(tc.tile_pool(name="psS", bufs=2, space="PSUM"))
        psAcc = ctx.enter_context(tc.tile_pool(name="psAcc", bufs=2, space="PSUM"))
        psD = ctx.enter_context(tc.tile_pool(name="psD", bufs=2, space="PSUM"))
        psO = ctx.enter_context(tc.tile_pool(name="psO", bufs=2, space="PSUM"))

        # ---------------- Phase B+C per tq chunk ----------------
        for c in range(NCH):
            sl = slice(c * CH, (c + 1) * CH)
            nts = (CH // 128) * (c + 1)  # causal: ts tiles 0..nts-1
            for h in range(HPC):
                pa = psAcc.tile([128, CH], F32, tag="acc")
                pd = psD.tile([128, CH], F32, tag="den")
                for j in range(nts):
                    ss = psS.tile([128, CH], F32, tag="psS")
                    nc.tensor.matmul(ss[:], qk_t[4][:, j * 128:(j + 1) * 128],
                                     qk_t[h][:, sl], start=True, stop=True)
                    pr = probs_pool.tile([128, CH], F32R, tag="probs")
                    nc.scalar.activation(pr[:], ss[:], AF.Exp, scale=SCALING)
                    i = j - (CH // 128) * c
                    if i >= 0:
                        nc.vector.tensor_mul(pr[:], pr[:], mask_sb[:, i, :])
                    nc.tensor.matmul(pd[:], ones_sb[:], pr[:],
                                     start=(j == 0), stop=(j == nts - 1))
                    nc.tensor.matmul(pa[:], v_sb[:, j, :], pr[:],
                                     start=(j == 0), stop=(j == nts - 1))
                rden = rden_pool.tile([128, CH], F32, tag="rden")
                nc.vector.reciprocal(rden[:], pd[:])
                nc.vector.tensor_mul(attn_sb[h][:, sl], pa[:], rden[:])
            # Phase C for this chunk: outT[:, sl] = wo.T @ attn
            for mo in range(HID // 128):
                po = psO.tile([128, CH], F32, tag="po")
                for kk in range(HPC):
                    nc.tensor.matmul(po[:], wo_sb[:, kk, mo, :],
                                     attn_sb[kk][:, sl],
                                     start=(kk == 0), stop=(kk == HPC - 1))
                ob = out_pool.tile([128, CH], F32, tag="ob")
                if mo % 2 == 0:
                    nc.scalar.copy(ob[:], po[:])
                else:
                    nc.vector.tensor_copy(ob[:], po[:])
                nc.sync.dma_start(outT[mo * 128:(mo + 1) * 128, sl], ob[:])

    nc.compile()
    return nc


def get_nc():
    if "nc" not in _CACHE:
        _CACHE["nc"] = _build_nc()
    return _CACHE["nc"]


def prep_in_maps(hidden_states, cos, sin, Wq, Wk, Wv, Wo):
    hidT = np.ascontiguousarray(hidden_states.T)
    cosT = np.ascontiguousarray(cos.T)
    sinT = np.ascontiguousarray(sin.T)
    # signed rotate-half permutation (as lhsT): rot = rmatT.T @ x
    rmatT = np.zeros((D, D), dtype=np.float32)
    half = D // 2
    rmatT[np.arange(half) + half, np.arange(half)] = -1.0  # rot[d<64] = -x[d+64]
    rmatT[np.arange(half), np.arange(half) + half] = 1.0   # rot[d>=64] = x[d-64]
    # causal diag masks: mask[i][p][f] = 1 if i*128+p <= f
    i_idx = np.arange(4)[:, None, None] * 128
    p_idx = np.arange(128)[None, :, None]
    f_idx = np.arange(CH)[None, None, :]
    masks = ((i_idx + p_idx) <= f_idx).astype(np.float32)

    in_maps = []
    for c in range(NCORES):
        wqkv = np.concatenate([
            Wq[:, c * QW:(c + 1) * QW],
            Wk[:, c * D:(c + 1) * D],
            Wv[:, c * D:(c + 1) * D],
        ], axis=1)
        in_maps.append({
            "hidT": hidT,
            "wqkv": np.ascontiguousarray(wqkv),
            "wo": np.ascontiguousarray(Wo[c * QW:(c + 1) * QW, :]),
            "cosT": cosT,
            "sinT": sinT,
            "rmatT": rmatT,
            "onesd": np.ones((128, 128), dtype=np.float32),
            "identd": np.eye(128, dtype=np.float32),
            "masks": masks,
        })
    return in_maps


def postprocess(results):
    acc = results[0]["outT_p"].copy()
    for r in results[1:]:
        acc += r["outT_p"]
    return np.ascontiguousarray(acc.T).astype(np.float32)


def kernel(hidden_states, position_ids, cos, sin, Wq, Wk, Wv, Wo):
    from concourse.bass_utils import run_bass_kernel_spmd
    nc = get_nc()
    in_maps = prep_in_maps(hidden_states, cos, sin, Wq, Wk, Wv, Wo)
    res = run_bass_kernel_spmd(nc, in_maps, core_ids=list(range(NCORES)))
    return postprocess(res.results)


# revision 12
# speedup vs baseline: 1.1042x; 1.1042x over previous
"""Trainium2 Bass kernel for nn_LlamaAttention (T=2048, HID=4096, HQ=32, HKV=8, D=128).

Tensor-parallel over heads across 8 NeuronCores: core c owns q-heads 4c..4c+3 and
kv-head c (GQA group size 4 == heads-per-core, so attention is fully core-local).
Wo is row-sharded; each core computes a partial [T, HID] output (transposed) and
the host sums the 8 partials. No device collectives.

Self-contained: hardcodes all shapes; builds the Bass kernel once per process.
"""
import numpy as np

T, HID, HQ, HKV, D = 2048, 4096, 32, 8, 128
NCORES = 8
HPC = HQ // NCORES            # 4 q heads per core
QW = HPC * D                  # 512 q columns per core
KO = HID // 128               # 32 k-tiles
ACH = 256                     # phase A T-chunk width (f32r needs N>=256)
NACH = T // ACH               # 8
CH = 512                      # attention tq chunk width
NCH = T // CH                 # 4
SCALING = float(D) ** -0.5

_CACHE = {}


def _build_nc():
    import concourse.mybir as mybir
    import concourse.tile as tile
    from concourse import bacc
    from contextlib import ExitStack

    F32 = mybir.dt.float32
    F32R = mybir.dt.float32r
    AF = mybir.ActivationFunctionType

    nc = bacc.Bacc("TRN2", target_bir_lowering=False, debug=False,
                   dynamic_dma_scratch_size=2048)

    hidT = nc.dram_tensor("hidT", [HID, T], F32R, kind="ExternalInput")
    wqkv = nc.dram_tensor("wqkv", [HID, QW + 2 * D], F32R, kind="ExternalInput")
    wo = nc.dram_tensor("wo", [QW, HID], F32R, kind="ExternalInput")
    cosT = nc.dram_tensor("cosT", [D, T], F32, kind="ExternalInput")
    sinT = nc.dram_tensor("sinT", [D, T], F32, kind="ExternalInput")
    rmatT = nc.dram_tensor("rmatT", [D, D], F32R, kind="ExternalInput")
    onesd = nc.dram_tensor("onesd", [128, 128], F32R, kind="ExternalInput")
    identd = nc.dram_tensor("identd", [128, 128], F32R, kind="ExternalInput")
    masks = nc.dram_tensor("masks", [4, 128, CH], F32, kind="ExternalInput")
    outT = nc.dram_tensor("outT_p", [HID, T], F32, kind="ExternalOutput")

    with tile.TileContext(nc) as tc, ExitStack() as ctx:
        consts = ctx.enter_context(tc.tile_pool(name="consts", bufs=1))
        ones_sb = consts.tile([128, 128], F32R)
        nc.sync.dma_start(ones_sb[:], onesd[:, :])
        ident = consts.tile([128, 128], F32R)
        nc.sync.dma_start(ident[:], identd[:, :])
        rmatT_sb = consts.tile([128, 128], F32R)
        nc.sync.dma_start(rmatT_sb[:], rmatT[:, :])

        qkv_pool = ctx.enter_context(tc.tile_pool(name="qkvT", bufs=1))
        # 5 blocks of qkvT: m=0..3 q heads, m=4 kT (v goes via staging tiles)
        qk_t = [qkv_pool.tile([128, T], F32R, name=f"qkvT{m}") for m in range(5)]
        v_sb = qkv_pool.tile([128, T // 128, 128], F32R, name="v_sb")  # [ts, blk, D]

        # ---------------- Phase A: qkvT = wqkv.T @ hidT, rope fused ----------
        with tc.tile_pool(name="wqkv", bufs=1) as wp, \
             tc.tile_pool(name="hid", bufs=2) as hp, \
             tc.tile_pool(name="vstage", bufs=2) as vsp, \
             tc.tile_pool(name="cs", bufs=4) as csp, \
             tc.tile_pool(name="ropetmp", bufs=2) as tmp_pool, \
             tc.tile_pool(name="psA", bufs=3, space="PSUM") as psA, \
             tc.tile_pool(name="psT", bufs=2, space="PSUM") as psT, \
             tc.tile_pool(name="psR", bufs=2, space="PSUM") as psR:
            hid3 = hidT.rearrange("(ko p) t -> p ko t", p=128)
            w3 = wqkv.rearrange("(ko p) m -> p ko m", p=128)
            # first hid chunk before weights so compute starts ASAP
            h_tiles = {}
            h_tiles[0] = hp.tile([128, KO, ACH], F32R, tag="hid", name="hid0")
            w_sb = [wp.tile([128, KO, 128], F32R, name=f"w{m}") for m in range(6)]
            KH = KO // 2
            nc.sync.dma_start(h_tiles[0][:, :KH], hid3[:, :KH, 0:ACH])
            nc.sync.dma_start(w_sb[0][:, :KH], w3[:, :KH, 0:128])
            nc.sync.dma_start(h_tiles[0][:, KH:], hid3[:, KH:, 0:ACH])
            nc.sync.dma_start(w_sb[0][:, KH:], w3[:, KH:, 0:128])
            for m in range(1, 6):
                nc.sync.dma_start(w_sb[m][:], w3[:, :, m * 128:(m + 1) * 128])
            for ch in range(NACH):
                asl = slice(ch * ACH, (ch + 1) * ACH)
                if ch not in h_tiles:
                    h_tiles[ch] = hp.tile([128, KO, ACH], F32R, tag="hid", name=f"hid{ch}")
                    nc.sync.dma_start(h_tiles[ch][:], hid3[:, :, asl])
                h_sb = h_tiles[ch]
                cos_ch = csp.tile([128, ACH], F32, tag="cos")
                nc.sync.dma_start(cos_ch[:], cosT[:, asl])
                sin_ch = csp.tile([128, ACH], F32, tag="sin")
                nc.sync.dma_start(sin_ch[:], sinT[:, asl])
                for m in range(6):
                    ps = psA.tile([128, ACH], F32, tag="psA")
                    for k in range(KO):
                        nc.tensor.matmul(
                            ps[:], w_sb[m][:, k, :], h_sb[:, k, :],
                            start=(k == 0), stop=(k == KO - 1))
                    if m < 5:
                        nc.vector.tensor_copy(qk_t[m][:, asl], ps[:])
                        # fused RoPE on this chunk: x = x*cos + (rmatT.T@x)*sin
                        rp = psR.tile([128, ACH], F32, tag="psR")
                        nc.tensor.matmul(rp[:], rmatT_sb[:], qk_t[m][:, asl],
                                         start=True, stop=True)
                        tmp = tmp_pool.tile([128, ACH], F32, tag="tmp")
                        nc.vector.tensor_mul(tmp[:], rp[:], sin_ch[:])
                        nc.vector.tensor_mul(qk_t[m][:, asl], qk_t[m][:, asl], cos_ch[:])
                        nc.vector.tensor_add(qk_t[m][:, asl], qk_t[m][:, asl], tmp[:])
                    else:
                        vstage = vsp.tile([128, ACH], F32R, tag="vst")
                        nc.vector.tensor_copy(vstage[:], ps[:])
                        for jj in range(ACH // 128):
                            j = ch * (ACH // 128) + jj
                            pst = psT.tile([128, 128], F32R, tag="psT")
                            nc.tensor.transpose(
                                pst[:], vstage[:, jj * 128:(jj + 1) * 128], ident[:])
                            nc.vector.tensor_copy(v_sb[:, j, :], pst[:])

        # ---------------- Phase B consts ----------------
        bconsts = ctx.enter_context(tc.tile_pool(name="bconsts", bufs=1))
        mask_sb = bconsts.tile([128, 4, CH], F32)
        nc.sync.dma_start(mask_sb[:], masks.rearrange("i p f -> p i f"))

        wop = ctx.enter_context(tc.tile_pool(name="wo", bufs=1))
        wo_sb = wop.tile([128, HPC, HID // 128, 128], F32R)
        nc.sync.dma_start(
            wo_sb[:], wo.rearrange("(kk p) (mo q) -> p kk mo q", p=128, q=128))

        probs_pool = ctx.enter_context(tc.tile_pool(name="probs", bufs=5))
        attn_pool = ctx.enter_context(tc.tile_pool(name="attnT", bufs=1))
        attn_sb = [attn_pool.tile([128, T], F32R, name=f"attnT{h}") for h in range(HPC)]
        rden_pool = ctx.enter_context(tc.tile_pool(name="rden", bufs=2))
        out_pool = ctx.enter_context(tc.tile_pool(name="outstage", bufs=4))
        psS = ctx.enter_context(tc.tile_pool(name="psS", bufs=2, space="PSUM"))
        psAcc = ctx.enter_context(tc.tile_pool(name="psAcc", bufs=2, space="PSUM"))
        psD = ctx.enter_context(tc.tile_pool(name="psD", bufs=2, space="PSUM"))
        psO = ctx.enter_context(tc.tile_pool(name="psO", bufs=2, space="PSUM"))

        # ---------------- Phase B+C per tq chunk ----------------
        for c in range(NCH):
            sl = slice(c * CH, (c + 1) * CH)
            nts = (CH // 128) * (c + 1)  # causal: ts tiles 0..nts-1
            for h in range(HPC):
                pa = psAcc.tile([128, CH], F32, tag="acc")
                pd = psD.tile([1, CH], F32, tag="den")
                for j in range(nts):
                    ss = psS.tile([128, CH], F32, tag="psS")
                    nc.tensor.matmul(ss[:], qk_t[4][:, j * 128:(j + 1) * 128],
                                     qk_t[h][:, sl], start=True, stop=True)
                    pr = probs_pool.tile([128, CH], F32R, tag="probs")
                    nc.scalar.activation(pr[:], ss[:], AF.Exp, scale=SCALING)
                    i = j - (CH // 128) * c
                    if i >= 0:
                        nc.vector.tensor_mul(pr[:], pr[:], mask_sb[:, i, :])
                    nc.tensor.matmul(pd[:], ones_sb[:, 0:1], pr[:],
                                     start=(j == 0), stop=(j == nts - 1))
                    nc.tensor.matmul(pa[:], v_sb[:, j, :], pr[:],
                                     start=(j == 0), stop=(j == nts - 1))
                rden = rden_pool.tile([1, CH], F32, tag="rden")
                nc.vector.reciprocal_approx_fast(out=rden[:], in_=pd[:])
                rden_bc = rden_pool.tile([128, CH], F32, tag="rdenbc")
                nc.gpsimd.partition_broadcast(rden_bc[:], rden[:])
                nc.vector.tensor_mul(attn_sb[h][:, sl], pa[:], rden_bc[:])
            # Phase C for this chunk: outT[:, sl] = wo.T @ attn
            for mo in range(HID // 128):
                po = psO.tile([128, CH], F32, tag="po")
                for kk in range(HPC):
                    nc.tensor.matmul(po[:], wo_sb[:, kk, mo, :],
                                     attn_sb[kk][:, sl],
                                     start=(kk == 0), stop=(kk == HPC - 1))
                ob = out_pool.tile([128, CH], F32, tag="ob")
                if mo % 2 == 0:
                    nc.scalar.copy(ob[:], po[:])
                else:
                    nc.vector.tensor_copy(ob[:], po[:])
                nc.sync.dma_start(outT[mo * 128:(mo + 1) * 128, sl], ob[:])

    nc.compile()
    return nc


def get_nc():
    if "nc" not in _CACHE:
        _CACHE["nc"] = _build_nc()
    return _CACHE["nc"]


def prep_in_maps(hidden_states, cos, sin, Wq, Wk, Wv, Wo):
    hidT = np.ascontiguousarray(hidden_states.T)
    cosT = np.ascontiguousarray(cos.T)
    sinT = np.ascontiguousarray(sin.T)
    # signed rotate-half permutation (as lhsT): rot = rmatT.T @ x
    rmatT = np.zeros((D, D), dtype=np.float32)
    half = D // 2
    rmatT[np.arange(half) + half, np.arange(half)] = -1.0  # rot[d<64] = -x[d+64]
    rmatT[np.arange(half), np.arange(half) + half] = 1.0   # rot[d>=64] = x[d-64]
    # causal diag masks: mask[i][p][f] = 1 if i*128+p <= f
    i_idx = np.arange(4)[:, None, None] * 128
    p_idx = np.arange(128)[None, :, None]
    f_idx = np.arange(CH)[None, None, :]
    masks = ((i_idx + p_idx) <= f_idx).astype(np.float32)

    in_maps = []
    for c in range(NCORES):
        wqkv = np.concatenate([
            Wq[:, c * QW:(c + 1) * QW],
            Wk[:, c * D:(c + 1) * D],
            Wv[:, c * D:(c + 1) * D],
        ], axis=1)
        in_maps.append({
            "hidT": hidT,
            "wqkv": np.ascontiguousarray(wqkv),
            "wo": np.ascontiguousarray(Wo[c * QW:(c + 1) * QW, :]),
            "cosT": cosT,
            "sinT": sinT,
            "rmatT": rmatT,
            "onesd": np.ones((128, 128), dtype=np.float32),
            "identd": np.eye(128, dtype=np.float32),
            "masks": masks,
        })
    return in_maps


def postprocess(results):
    acc = results[0]["outT_p"].copy()
    for r in results[1:]:
        acc += r["outT_p"]
    return np.ascontiguousarray(acc.T).astype(np.float32)


def kernel(hidden_states, position_ids, cos, sin, Wq, Wk, Wv, Wo):
    from concourse.bass_utils import run_bass_kernel_spmd
    nc = get_nc()
    in_maps = prep_in_maps(hidden_states, cos, sin, Wq, Wk, Wv, Wo)
    res = run_bass_kernel_spmd(nc, in_maps, core_ids=list(range(NCORES)))
    return postprocess(res.results)


# revision 14
# speedup vs baseline: 1.1975x; 1.0845x over previous
"""Trainium2 Bass kernel for nn_LlamaAttention (T=2048, HID=4096, HQ=32, HKV=8, D=128).

Tensor-parallel over heads across 8 NeuronCores: core c owns q-heads 4c..4c+3 and
kv-head c (GQA group size 4 == heads-per-core, so attention is fully core-local).
Wo is row-sharded; each core computes a partial [T, HID] output (transposed) and
the host sums the 8 partials. No device collectives.

Self-contained: hardcodes all shapes; builds the Bass kernel once per process.
"""
import numpy as np

T, HID, HQ, HKV, D = 2048, 4096, 32, 8, 128
NCORES = 8
HPC = HQ // NCORES            # 4 q heads per core
QW = HPC * D                  # 512 q columns per core
KO = HID // 128               # 32 k-tiles
ACH = 256                     # phase A T-chunk width (f32r needs N>=256)
NACH = T // ACH               # 8
CH = 512                      # attention tq chunk width
NCH = T // CH                 # 4
SCALING = float(D) ** -0.5

_CACHE = {}


def _build_nc():
    import concourse.mybir as mybir
    import concourse.tile as tile
    from concourse import bacc
    from contextlib import ExitStack

    F32 = mybir.dt.float32
    F32R = mybir.dt.float32r
    AF = mybir.ActivationFunctionType

    nc = bacc.Bacc("TRN2", target_bir_lowering=False, debug=False,
                   dynamic_dma_scratch_size=2048)

    hidT = nc.dram_tensor("hidT", [HID, T], F32R, kind="ExternalInput")
    wqkv = nc.dram_tensor("wqkv", [HID, QW + 2 * D], F32R, kind="ExternalInput")
    wo = nc.dram_tensor("wo", [QW, HID], F32R, kind="ExternalInput")
    cosT = nc.dram_tensor("cosT", [D, T], F32, kind="ExternalInput")
    sinT = nc.dram_tensor("sinT", [D, T], F32, kind="ExternalInput")
    rmatT = nc.dram_tensor("rmatT", [D, D], F32R, kind="ExternalInput")
    onesd = nc.dram_tensor("onesd", [128, 128], F32R, kind="ExternalInput")
    identd = nc.dram_tensor("identd", [128, 128], F32R, kind="ExternalInput")
    masks = nc.dram_tensor("masks", [4, 128, CH], F32, kind="ExternalInput")
    outT = nc.dram_tensor("outT_p", [HID, T], F32, kind="ExternalOutput")

    with tile.TileContext(nc) as tc, ExitStack() as ctx:
        consts = ctx.enter_context(tc.tile_pool(name="consts", bufs=1))
        ones_sb = consts.tile([128, 128], F32R)
        nc.sync.dma_start(ones_sb[:], onesd[:, :])
        ident = consts.tile([128, 128], F32R)
        nc.sync.dma_start(ident[:], identd[:, :])
        rmatT_sb = consts.tile([128, 128], F32R)
        nc.sync.dma_start(rmatT_sb[:], rmatT[:, :])

        qkv_pool = ctx.enter_context(tc.tile_pool(name="qkvT", bufs=1))
        # 5 blocks of qkvT: m=0..3 q heads, m=4 kT (v goes via staging tiles)
        qk_t = [qkv_pool.tile([128, T], F32R, name=f"qkvT{m}") for m in range(6)]
        v_sb = qkv_pool.tile([128, T // 128, 128], F32R, name="v_sb")  # [ts, blk, D]

        # ---- Phase A: qkvT = wqkv.T @ hidT, 2-pass K-split, rope fused ----
        # N=512 matmuls (halves LDWEIGHTS count vs N=256; fp32r LDW is 2-pass
        # ~194ns, so N=256 would be LDW-bound). K split into two 16-tile
        # halves so W-half + one 512-wide hid chunk fit in SBUF; pass 2
        # accumulates into qkvT via DVE add.
        AC2 = 512
        NAC2 = T // AC2
        KH = KO // 2
        with tc.tile_pool(name="wqkv", bufs=9) as wp, \
             tc.tile_pool(name="hid", bufs=2) as hp, \
             tc.tile_pool(name="cs", bufs=4) as csp, \
             tc.tile_pool(name="ropetmp", bufs=2) as tmp_pool, \
             tc.tile_pool(name="psA", bufs=3, space="PSUM") as psA, \
             tc.tile_pool(name="psT", bufs=2, space="PSUM") as psT, \
             tc.tile_pool(name="psR", bufs=2, space="PSUM") as psR:
            hid3 = hidT.rearrange("(ko p) t -> p ko t", p=128)
            w3 = wqkv.rearrange("(ko p) m -> p ko m", p=128)
            w_sb = {}

            def load_w(kh, m, split=False):
                wt = wp.tile([128, KH, 128], F32R, tag="w", name=f"w{kh}_{m}")
                ks0 = kh * KH
                if split:
                    nc.sync.dma_start(wt[:, :KH // 2],
                                      w3[:, ks0:ks0 + KH // 2, m * 128:(m + 1) * 128])
                    nc.sync.dma_start(wt[:, KH // 2:],
                                      w3[:, ks0 + KH // 2:ks0 + KH, m * 128:(m + 1) * 128])
                else:
                    nc.sync.dma_start(wt[:], w3[:, ks0:ks0 + KH, m * 128:(m + 1) * 128])
                w_sb[(kh, m)] = wt

            def load_hid(kh, ch, split=False):
                ht = hp.tile([128, KH, AC2], F32R, tag="hid", name=f"h{kh}_{ch}")
                ks0 = kh * KH
                asl = slice(ch * AC2, (ch + 1) * AC2)
                if split:
                    nc.sync.dma_start(ht[:, :KH // 2], hid3[:, ks0:ks0 + KH // 2, asl])
                    nc.sync.dma_start(ht[:, KH // 2:], hid3[:, ks0 + KH // 2:ks0 + KH, asl])
                else:
                    nc.sync.dma_start(ht[:], hid3[:, ks0:ks0 + KH, asl])
                return ht

            # startup: interleave first hid chunk + first W so PE starts ASAP
            h_cur = load_hid(0, 0, split=True)
            load_w(0, 0, split=True)
            for m in range(1, 6):
                load_w(0, m)
            for m in range(6):
                load_w(1, m)
            for kh in range(2):
                for ch in range(NAC2):
                    asl = slice(ch * AC2, (ch + 1) * AC2)
                    if not (kh == 0 and ch == 0):
                        h_cur = load_hid(kh, ch)
                    if kh == 1:
                        cos_ch = csp.tile([128, AC2], F32, tag="cos")
                        nc.sync.dma_start(cos_ch[:], cosT[:, asl])
                        sin_ch = csp.tile([128, AC2], F32, tag="sin")
                        nc.sync.dma_start(sin_ch[:], sinT[:, asl])
                    for m in range(6):
                        ps = psA.tile([128, AC2], F32, tag="psA")
                        for k in range(KH):
                            nc.tensor.matmul(
                                ps[:], w_sb[(kh, m)][:, k, :], h_cur[:, k, :],
                                start=(k == 0), stop=(k == KH - 1))
                        if kh == 0:
                            nc.vector.tensor_copy(qk_t[m][:, asl], ps[:])
                            continue
                        nc.vector.tensor_add(qk_t[m][:, asl], qk_t[m][:, asl], ps[:])
                        if m < 5:
                            # fused RoPE: x = x*cos + (rmatT.T@x)*sin
                            rp = psR.tile([128, AC2], F32, tag="psR")
                            nc.tensor.matmul(rp[:], rmatT_sb[:], qk_t[m][:, asl],
                                             start=True, stop=True)
                            tmp = tmp_pool.tile([128, AC2], F32, tag="tmp")
                            nc.vector.tensor_mul(tmp[:], rp[:], sin_ch[:])
                            nc.vector.tensor_mul(qk_t[m][:, asl], qk_t[m][:, asl],
                                                 cos_ch[:])
                            nc.vector.tensor_add(qk_t[m][:, asl], qk_t[m][:, asl],
                                                 tmp[:])
                        else:
                            for jj in range(AC2 // 128):
                                j = ch * (AC2 // 128) + jj
                                pst = psT.tile([128, 128], F32R, tag="psT")
                                nc.tensor.transpose(
                                    pst[:],
                                    qk_t[5][:, j * 128:(j + 1) * 128],
                                    ident[:])
                                nc.vector.tensor_copy(v_sb[:, j, :], pst[:])

        # ---------------- Phase B consts ----------------
        bconsts = ctx.enter_context(tc.tile_pool(name="bconsts", bufs=1))
        mask_sb = bconsts.tile([128, 4, CH], F32)
        nc.sync.dma_start(mask_sb[:], masks.rearrange("i p f -> p i f"))

        wop = ctx.enter_context(tc.tile_pool(name="wo", bufs=1))
        wo_sb = wop.tile([128, HPC, HID // 128, 128], F32R)
        nc.sync.dma_start(
            wo_sb[:], wo.rearrange("(kk p) (mo q) -> p kk mo q", p=128, q=128))

        probs_pool = ctx.enter_context(tc.tile_pool(name="probs", bufs=5))
        attn_pool = ctx.enter_context(tc.tile_pool(name="attnT", bufs=1))
        attn_sb = [attn_pool.tile([128, T], F32R, name=f"attnT{h}") for h in range(HPC)]
        rden_pool = ctx.enter_context(tc.tile_pool(name="rden", bufs=2))
        out_pool = ctx.enter_context(tc.tile_pool(name="outstage", bufs=4))
        psS = ctx.enter_context(tc.tile_pool(name="psS", bufs=2, space="PSUM"))
        psAcc = ctx.enter_context(tc.tile_pool(name="psAcc", bufs=2, space="PSUM"))
        psD = ctx.enter_context(tc.tile_pool(name="psD", bufs=2, space="PSUM"))
        psO = ctx.enter_context(tc.tile_pool(name="psO", bufs=2, space="PSUM"))

        # ---------------- Phase B+C per tq chunk (C deferred one chunk) ---
        def phase_c(c):
            sl = slice(c * CH, (c + 1) * CH)
            for mo in range(HID // 128):
                po = psO.tile([128, CH], F32, tag="po")
                for kk in range(HPC):
                    nc.tensor.matmul(po[:], wo_sb[:, kk, mo, :],
                                     attn_sb[kk][:, sl],
                                     start=(kk == 0), stop=(kk == HPC - 1))
                ob = out_pool.tile([128, CH], F32, tag="ob")
                if mo % 2 == 0:
                    nc.scalar.copy(ob[:], po[:])
                else:
                    nc.vector.tensor_copy(ob[:], po[:])
                nc.sync.dma_start(outT[mo * 128:(mo + 1) * 128, sl], ob[:])

        for c in range(NCH):
            sl = slice(c * CH, (c + 1) * CH)
            nts = (CH // 128) * (c + 1)  # causal: ts tiles 0..nts-1
            for h in range(HPC):
                pa = psAcc.tile([128, CH], F32, tag="acc")
                pd = psD.tile([1, CH], F32, tag="den")
                for j in range(nts):
                    ss = psS.tile([128, CH], F32, tag="psS")
                    nc.tensor.matmul(ss[:], qk_t[4][:, j * 128:(j + 1) * 128],
                                     qk_t[h][:, sl], start=True, stop=True)
                    pr = probs_pool.tile([128, CH], F32R, tag="probs")
                    nc.scalar.activation(pr[:], ss[:], AF.Exp, scale=SCALING)
                    i = j - (CH // 128) * c
                    if i >= 0:
                        nc.vector.tensor_mul(pr[:], pr[:], mask_sb[:, i, :])
                    nc.tensor.matmul(pd[:], ones_sb[:, 0:1], pr[:],
                                     start=(j == 0), stop=(j == nts - 1))
                    nc.tensor.matmul(pa[:], v_sb[:, j, :], pr[:],
                                     start=(j == 0), stop=(j == nts - 1))
                rden = rden_pool.tile([1, CH], F32, tag="rden")
                nc.vector.reciprocal_approx_fast(out=rden[:], in_=pd[:])
                rden_bc = rden_pool.tile([128, CH], F32, tag="rdenbc")
                nc.gpsimd.partition_broadcast(rden_bc[:], rden[:])
                nc.vector.tensor_mul(attn_sb[h][:, sl], pa[:], rden_bc[:])
            if c >= 1:
                phase_c(c - 1)
        phase_c(NCH - 1)

    nc.compile()
    return nc


def get_nc():
    if "nc" not in _CACHE:
        _CACHE["nc"] = _build_nc()
    return _CACHE["nc"]


def prep_in_maps(hidden_states, cos, sin, Wq, Wk, Wv, Wo):
    hidT = np.ascontiguousarray(hidden_states.T)
    cosT = np.ascontiguousarray(cos.T)
    sinT = np.ascontiguousarray(sin.T)
    # signed rotate-half permutation (as lhsT): rot = rmatT.T @ x
    rmatT = np.zeros((D, D), dtype=np.float32)
    half = D // 2
    rmatT[np.arange(half) + half, np.arange(half)] = -1.0  # rot[d<64] = -x[d+64]
    rmatT[np.arange(half), np.arange(half) + half] = 1.0   # rot[d>=64] = x[d-64]
    # causal diag masks: mask[i][p][f] = 1 if i*128+p <= f
    i_idx = np.arange(4)[:, None, None] * 128
    p_idx = np.arange(128)[None, :, None]
    f_idx = np.arange(CH)[None, None, :]
    masks = ((i_idx + p_idx) <= f_idx).astype(np.float32)

    in_maps = []
    for c in range(NCORES):
        wqkv = np.concatenate([
            Wq[:, c * QW:(c + 1) * QW],
            Wk[:, c * D:(c + 1) * D],
            Wv[:, c * D:(c + 1) * D],
        ], axis=1)
        in_maps.append({
            "hidT": hidT,
            "wqkv": np.ascontiguousarray(wqkv),
            "wo": np.ascontiguousarray(Wo[c * QW:(c + 1) * QW, :]),
            "cosT": cosT,
            "sinT": sinT,
            "rmatT": rmatT,
            "onesd": np.ones((128, 128), dtype=np.float32),
            "identd": np.eye(128, dtype=np.float32),
            "masks": masks,
        })
    return in_maps


def postprocess(results):
    acc = results[0]["outT_p"].copy()
    for r in results[1:]:
        acc += r["outT_p"]
    return np.ascontiguousarray(acc.T).astype(np.float32)


def kernel(hidden_states, position_ids, cos, sin, Wq, Wk, Wv, Wo):
    from concourse.bass_utils import run_bass_kernel_spmd
    nc = get_nc()
    in_maps = prep_in_maps(hidden_states, cos, sin, Wq, Wk, Wv, Wo)
    res = run_bass_kernel_spmd(nc, in_maps, core_ids=list(range(NCORES)))
    return postprocess(res.results)


# revision 15
# speedup vs baseline: 1.2521x; 1.0456x over previous
"""Trainium2 Bass kernel for nn_LlamaAttention (T=2048, HID=4096, HQ=32, HKV=8, D=128).

Tensor-parallel over heads across 8 NeuronCores: core c owns q-heads 4c..4c+3 and
kv-head c (GQA group size 4 == heads-per-core, so attention is fully core-local).
Wo is row-sharded; each core computes a partial [T, HID] output (transposed) and
the host sums the 8 partials. No device collectives.

Self-contained: hardcodes all shapes; builds the Bass kernel once per process.
"""
import numpy as np

T, HID, HQ, HKV, D = 2048, 4096, 32, 8, 128
NCORES = 8
HPC = HQ // NCORES            # 4 q heads per core
QW = HPC * D                  # 512 q columns per core
KO = HID // 128               # 32 k-tiles
ACH = 256                     # phase A T-chunk width (f32r needs N>=256)
NACH = T // ACH               # 8
CH = 512                      # attention tq chunk width
NCH = T // CH                 # 4
SCALING = float(D) ** -0.5

_CACHE = {}


def _build_nc():
    import concourse.mybir as mybir
    import concourse.tile as tile
    from concourse import bacc
    from contextlib import ExitStack

    F32 = mybir.dt.float32
    F32R = mybir.dt.float32r
    AF = mybir.ActivationFunctionType

    nc = bacc.Bacc("TRN2", target_bir_lowering=False, debug=False,
                   dynamic_dma_scratch_size=2048)

    hidT = nc.dram_tensor("hidT", [HID, T], F32R, kind="ExternalInput")
    wqkv = nc.dram_tensor("wqkv", [HID, QW + 2 * D], F32R, kind="ExternalInput")
    wo = nc.dram_tensor("wo", [QW, HID], F32R, kind="ExternalInput")
    cosT = nc.dram_tensor("cosT", [D, T], F32, kind="ExternalInput")
    sinT = nc.dram_tensor("sinT", [D, T], F32, kind="ExternalInput")
    rmatT = nc.dram_tensor("rmatT", [D, D], F32R, kind="ExternalInput")
    onesd = nc.dram_tensor("onesd", [128, 128], F32R, kind="ExternalInput")
    identd = nc.dram_tensor("identd", [128, 128], F32R, kind="ExternalInput")
    masks = nc.dram_tensor("masks", [4, 128, CH], F32, kind="ExternalInput")
    outT = nc.dram_tensor("outT_p", [HID, T], F32, kind="ExternalOutput")

    with tile.TileContext(nc) as tc, ExitStack() as ctx:
        consts = ctx.enter_context(tc.tile_pool(name="consts", bufs=1))
        ones_sb = consts.tile([128, 128], F32R)
        nc.sync.dma_start(ones_sb[:], onesd[:, :])
        ident = consts.tile([128, 128], F32R)
        nc.sync.dma_start(ident[:], identd[:, :])
        rmatT_sb = consts.tile([128, 128], F32R)
        nc.sync.dma_start(rmatT_sb[:], rmatT[:, :])

        qkv_pool = ctx.enter_context(tc.tile_pool(name="qkvT", bufs=1))
        # 5 blocks of qkvT: m=0..3 q heads, m=4 kT (v goes via staging tiles)
        qk_t = [qkv_pool.tile([128, T], F32R, name=f"qkvT{m}") for m in range(6)]
        v_sb = qkv_pool.tile([128, T // 128, 128], F32R, name="v_sb")  # [ts, blk, D]

        # ---- Phase A: qkvT = wqkv.T @ hidT, 2-pass K-split, rope fused ----
        # N=512 matmuls (halves LDWEIGHTS count vs N=256; fp32r LDW is 2-pass
        # ~194ns, so N=256 would be LDW-bound). K split into two 16-tile
        # halves so W-half + one 512-wide hid chunk fit in SBUF; pass 2
        # accumulates into qkvT via DVE add.
        AC2 = 512
        NAC2 = T // AC2
        KH = KO // 2
        with tc.tile_pool(name="wqkv", bufs=9) as wp, \
             tc.tile_pool(name="hid", bufs=2) as hp, \
             tc.tile_pool(name="cs", bufs=4) as csp, \
             tc.tile_pool(name="ropetmp", bufs=2) as tmp_pool, \
             tc.tile_pool(name="psA", bufs=3, space="PSUM") as psA, \
             tc.tile_pool(name="psT", bufs=2, space="PSUM") as psT, \
             tc.tile_pool(name="psR", bufs=2, space="PSUM") as psR:
            hid3 = hidT.rearrange("(ko p) t -> p ko t", p=128)
            w3 = wqkv.rearrange("(ko p) m -> p ko m", p=128)
            w_sb = {}

            def load_w(kh, m, split=False):
                wt = wp.tile([128, KH, 128], F32R, tag="w", name=f"w{kh}_{m}")
                ks0 = kh * KH
                if split:
                    nc.sync.dma_start(wt[:, :KH // 2],
                                      w3[:, ks0:ks0 + KH // 2, m * 128:(m + 1) * 128])
                    nc.sync.dma_start(wt[:, KH // 2:],
                                      w3[:, ks0 + KH // 2:ks0 + KH, m * 128:(m + 1) * 128])
                else:
                    nc.sync.dma_start(wt[:], w3[:, ks0:ks0 + KH, m * 128:(m + 1) * 128])
                w_sb[(kh, m)] = wt

            def load_hid(kh, ch, split=False):
                ht = hp.tile([128, KH, AC2], F32R, tag="hid", name=f"h{kh}_{ch}")
                ks0 = kh * KH
                asl = slice(ch * AC2, (ch + 1) * AC2)
                if split:
                    nc.sync.dma_start(ht[:, :KH // 2], hid3[:, ks0:ks0 + KH // 2, asl])
                    nc.sync.dma_start(ht[:, KH // 2:], hid3[:, ks0 + KH // 2:ks0 + KH, asl])
                else:
                    nc.sync.dma_start(ht[:], hid3[:, ks0:ks0 + KH, asl])
                return ht

            # startup: interleave first hid chunk + first W so PE starts ASAP
            h_cur = load_hid(0, 0, split=True)
            load_w(0, 0, split=True)
            for m in range(1, 6):
                load_w(0, m)
            for kh in range(2):
                for ch in range(NAC2):
                    asl = slice(ch * AC2, (ch + 1) * AC2)
                    if not (kh == 0 and ch == 0):
                        h_cur = load_hid(kh, ch)
                    if kh == 0 and ch >= 1:
                        # stagger second K-half weight loads behind hid chunks
                        load_w(1, 2 * (ch - 1))
                        load_w(1, 2 * (ch - 1) + 1)
                    if kh == 1:
                        cos_ch = csp.tile([128, AC2], F32, tag="cos")
                        nc.sync.dma_start(cos_ch[:], cosT[:, asl])
                        sin_ch = csp.tile([128, AC2], F32, tag="sin")
                        nc.sync.dma_start(sin_ch[:], sinT[:, asl])
                    for m in range(6):
                        ps = psA.tile([128, AC2], F32, tag="psA")
                        for k in range(KH):
                            nc.tensor.matmul(
                                ps[:], w_sb[(kh, m)][:, k, :], h_cur[:, k, :],
                                start=(k == 0), stop=(k == KH - 1))
                        if kh == 0:
                            nc.vector.tensor_copy(qk_t[m][:, asl], ps[:])
                            continue
                        nc.vector.tensor_add(qk_t[m][:, asl], qk_t[m][:, asl], ps[:])
                        if m < 5:
                            # fused RoPE: x = x*cos + (rmatT.T@x)*sin
                            rp = psR.tile([128, AC2], F32, tag="psR")
                            nc.tensor.matmul(rp[:], rmatT_sb[:], qk_t[m][:, asl],
                                             start=True, stop=True)
                            tmp = tmp_pool.tile([128, AC2], F32, tag="tmp")
                            nc.vector.tensor_mul(tmp[:], rp[:], sin_ch[:])
                            nc.vector.tensor_mul(qk_t[m][:, asl], qk_t[m][:, asl],
                                                 cos_ch[:])
                            nc.vector.tensor_add(qk_t[m][:, asl], qk_t[m][:, asl],
                                                 tmp[:])
                        else:
                            for jj in range(AC2 // 128):
                                j = ch * (AC2 // 128) + jj
                                pst = psT.tile([128, 128], F32R, tag="psT")
                                nc.tensor.transpose(
                                    pst[:],
                                    qk_t[5][:, j * 128:(j + 1) * 128],
                                    ident[:])
                                nc.vector.tensor_copy(v_sb[:, j, :], pst[:])

        # ---------------- Phase B consts ----------------
        bconsts = ctx.enter_context(tc.tile_pool(name="bconsts", bufs=1))
        mask_sb = bconsts.tile([128, 4, CH], F32)
        nc.sync.dma_start(mask_sb[:], masks.rearrange("i p f -> p i f"))

        wop = ctx.enter_context(tc.tile_pool(name="wo", bufs=1))
        wo_sb = wop.tile([128, HPC, HID // 128, 128], F32R)
        nc.sync.dma_start(
            wo_sb[:], wo.rearrange("(kk p) (mo q) -> p kk mo q", p=128, q=128))

        probs_pool = ctx.enter_context(tc.tile_pool(name="probs", bufs=5))
        attn_pool = ctx.enter_context(tc.tile_pool(name="attnT", bufs=1))
        attn_sb = [attn_pool.tile([128, T], F32R, name=f"attnT{h}") for h in range(HPC)]
        rden_pool = ctx.enter_context(tc.tile_pool(name="rden", bufs=2))
        out_pool = ctx.enter_context(tc.tile_pool(name="outstage", bufs=4))
        psS = ctx.enter_context(tc.tile_pool(name="psS", bufs=2, space="PSUM"))
        psAcc = ctx.enter_context(tc.tile_pool(name="psAcc", bufs=2, space="PSUM"))
        psD = ctx.enter_context(tc.tile_pool(name="psD", bufs=2, space="PSUM"))
        psO = ctx.enter_context(tc.tile_pool(name="psO", bufs=2, space="PSUM"))

        # ---------------- Phase B+C per tq chunk (C deferred one chunk) ---
        def phase_c(c):
            sl = slice(c * CH, (c + 1) * CH)
            for mo in range(HID // 128):
                po = psO.tile([128, CH], F32, tag="po")
                for kk in range(HPC):
                    nc.tensor.matmul(po[:], wo_sb[:, kk, mo, :],
                                     attn_sb[kk][:, sl],
                                     start=(kk == 0), stop=(kk == HPC - 1))
                ob = out_pool.tile([128, CH], F32, tag="ob")
                if mo % 2 == 0:
                    nc.scalar.copy(ob[:], po[:])
                else:
                    nc.vector.tensor_copy(ob[:], po[:])
                nc.sync.dma_start(outT[mo * 128:(mo + 1) * 128, sl], ob[:])

        for c in range(NCH):
            sl = slice(c * CH, (c + 1) * CH)
            nts = (CH // 128) * (c + 1)  # causal: ts tiles 0..nts-1
            for h in range(HPC):
                pa = psAcc.tile([128, CH], F32, tag="acc")
                pd = psD.tile([128, CH], F32, tag="den")
                for j in range(nts):
                    ss = psS.tile([128, CH], F32, tag="psS")
                    nc.tensor.matmul(ss[:], qk_t[4][:, j * 128:(j + 1) * 128],
                                     qk_t[h][:, sl], start=True, stop=True)
                    pr = probs_pool.tile([128, CH], F32R, tag="probs")
                    nc.scalar.activation(pr[:], ss[:], AF.Exp, scale=SCALING)
                    i = j - (CH // 128) * c
                    if i >= 0:
                        nc.vector.tensor_mul(pr[:], pr[:], mask_sb[:, i, :])
                    nc.tensor.matmul(pd[:], ones_sb[:], pr[:],
                                     start=(j == 0), stop=(j == nts - 1))
                    nc.tensor.matmul(pa[:], v_sb[:, j, :], pr[:],
                                     start=(j == 0), stop=(j == nts - 1))
                rden = rden_pool.tile([128, CH], F32, tag="rden")
                nc.vector.reciprocal_approx_fast(out=rden[:], in_=pd[:])
                nc.vector.tensor_mul(attn_sb[h][:, sl], pa[:], rden[:])
            if c >= 1:
                phase_c(c - 1)
        phase_c(NCH - 1)

    nc.compile()
    return nc


def get_nc():
    if "nc" not in _CACHE:
        _CACHE["nc"] = _build_nc()
    return _CACHE["nc"]


def prep_in_maps(hidden_states, cos, sin, Wq, Wk, Wv, Wo):
    hidT = np.ascontiguousarray(hidden_states.T)
    cosT = np.ascontiguousarray(cos.T)
    sinT = np.ascontiguousarray(sin.T)
    # signed rotate-half permutation (as lhsT): rot = rmatT.T @ x
    rmatT = np.zeros((D, D), dtype=np.float32)
    half = D // 2
    rmatT[np.arange(half) + half, np.arange(half)] = -1.0  # rot[d<64] = -x[d+64]
    rmatT[np.arange(half), np.arange(half) + half] = 1.0   # rot[d>=64] = x[d-64]
    # causal diag masks: mask[i][p][f] = 1 if i*128+p <= f
    i_idx = np.arange(4)[:, None, None] * 128
    p_idx = np.arange(128)[None, :, None]
    f_idx = np.arange(CH)[None, None, :]
    masks = ((i_idx + p_idx) <= f_idx).astype(np.float32)

    in_maps = []
    for c in range(NCORES):
        wqkv = np.concatenate([
            Wq[:, c * QW:(c + 1) * QW],
            Wk[:, c * D:(c + 1) * D],
            Wv[:, c * D:(c + 1) * D],
        ], axis=1)
        in_maps.append({
            "hidT": hidT,
            "wqkv": np.ascontiguousarray(wqkv),
            "wo": np.ascontiguousarray(Wo[c * QW:(c + 1) * QW, :]),
            "cosT": cosT,
            "sinT": sinT,
            "rmatT": rmatT,
            "onesd": np.ones((128, 128), dtype=np.float32),
            "identd": np.eye(128, dtype=np.float32),
            "masks": masks,
        })
    return in_maps


def postprocess(results):
    acc = results[0]["outT_p"].copy()
    for r in results[1:]:
        acc += r["outT_p"]
    return np.ascontiguousarray(acc.T).astype(np.float32)


def kernel(hidden_states, position_ids, cos, sin, Wq, Wk, Wv, Wo):
    from concourse.bass_utils import run_bass_kernel_spmd
    nc = get_nc()
    in_maps = prep_in_maps(hidden_states, cos, sin, Wq, Wk, Wv, Wo)
    res = run_bass_kernel_spmd(nc, in_maps, core_ids=list(range(NCORES)))
    return postprocess(res.results)


# revision 16
# speedup vs baseline: 1.2841x; 1.0255x over previous
"""Trainium2 Bass kernel for nn_LlamaAttention (T=2048, HID=4096, HQ=32, HKV=8, D=128).

Tensor-parallel over heads across 8 NeuronCores: core c owns q-heads 4c..4c+3 and
kv-head c (GQA group size 4 == heads-per-core, so attention is fully core-local).
Wo is row-sharded; each core computes a partial [T, HID] output (transposed) and
the host sums the 8 partials. No device collectives.

Self-contained: hardcodes all shapes; builds the Bass kernel once per process.
"""
import numpy as np

T, HID, HQ, HKV, D = 2048, 4096, 32, 8, 128
NCORES = 8
HPC = HQ // NCORES            # 4 q heads per core
QW = HPC * D                  # 512 q columns per core
KO = HID // 128               # 32 k-tiles
ACH = 256                     # phase A T-chunk width (f32r needs N>=256)
NACH = T // ACH               # 8
CH = 512                      # attention tq chunk width
NCH = T // CH                 # 4
SCALING = float(D) ** -0.5

_CACHE = {}


def _build_nc():
    import concourse.mybir as mybir
    import concourse.tile as tile
    from concourse import bacc
    from contextlib import ExitStack

    F32 = mybir.dt.float32
    F32R = mybir.dt.float32r
    AF = mybir.ActivationFunctionType

    nc = bacc.Bacc("TRN2", target_bir_lowering=False, debug=False,
                   dynamic_dma_scratch_size=2048)

    hidT = nc.dram_tensor("hidT", [HID, T], F32R, kind="ExternalInput")
    wqkv = nc.dram_tensor("wqkv", [HID, QW + 2 * D], F32R, kind="ExternalInput")
    wo = nc.dram_tensor("wo", [QW, HID], F32R, kind="ExternalInput")
    cosT = nc.dram_tensor("cosT", [D, T], F32, kind="ExternalInput")
    sinT = nc.dram_tensor("sinT", [D, T], F32, kind="ExternalInput")
    rmatT = nc.dram_tensor("rmatT", [D, D], F32R, kind="ExternalInput")
    onesd = nc.dram_tensor("onesd", [128, 128], F32R, kind="ExternalInput")
    identd = nc.dram_tensor("identd", [128, 128], F32R, kind="ExternalInput")
    masks = nc.dram_tensor("masks", [4, 128, CH], F32, kind="ExternalInput")
    outT = nc.dram_tensor("outT_p", [HID, T], F32, kind="ExternalOutput")

    with tile.TileContext(nc) as tc, ExitStack() as ctx:
        consts = ctx.enter_context(tc.tile_pool(name="consts", bufs=1))
        ones_sb = consts.tile([128, 128], F32R)
        ident = consts.tile([128, 128], F32R)
        rmatT_sb = consts.tile([128, 128], F32R)

        qkv_pool = ctx.enter_context(tc.tile_pool(name="qkvT", bufs=1))
        # 5 blocks of qkvT: m=0..3 q heads, m=4 kT (v goes via staging tiles)
        qk_t = [qkv_pool.tile([128, T], F32R, name=f"qkvT{m}") for m in range(6)]
        v_sb = qkv_pool.tile([128, T // 128, 128], F32R, name="v_sb")  # [ts, blk, D]

        # ---- Phase A: qkvT = wqkv.T @ hidT, 2-pass K-split, rope fused ----
        # N=512 matmuls (halves LDWEIGHTS count vs N=256; fp32r LDW is 2-pass
        # ~194ns, so N=256 would be LDW-bound). K split into two 16-tile
        # halves so W-half + one 512-wide hid chunk fit in SBUF; pass 2
        # accumulates into qkvT via DVE add.
        AC2 = 512
        NAC2 = T // AC2
        KH = KO // 2
        with tc.tile_pool(name="wqkv", bufs=9) as wp, \
             tc.tile_pool(name="hid", bufs=2) as hp, \
             tc.tile_pool(name="cs", bufs=4) as csp, \
             tc.tile_pool(name="ropetmp", bufs=2) as tmp_pool, \
             tc.tile_pool(name="psA", bufs=3, space="PSUM") as psA, \
             tc.tile_pool(name="psT", bufs=2, space="PSUM") as psT, \
             tc.tile_pool(name="psR", bufs=2, space="PSUM") as psR:
            hid3 = hidT.rearrange("(ko p) t -> p ko t", p=128)
            w3 = wqkv.rearrange("(ko p) m -> p ko m", p=128)
            w_sb = {}

            def load_w(kh, m, split=False):
                wt = wp.tile([128, KH, 128], F32R, tag="w", name=f"w{kh}_{m}")
                ks0 = kh * KH
                if split:
                    nc.sync.dma_start(wt[:, :KH // 2],
                                      w3[:, ks0:ks0 + KH // 2, m * 128:(m + 1) * 128])
                    nc.sync.dma_start(wt[:, KH // 2:],
                                      w3[:, ks0 + KH // 2:ks0 + KH, m * 128:(m + 1) * 128])
                else:
                    nc.sync.dma_start(wt[:], w3[:, ks0:ks0 + KH, m * 128:(m + 1) * 128])
                w_sb[(kh, m)] = wt

            def load_hid(kh, ch, split=False):
                ht = hp.tile([128, KH, AC2], F32R, tag="hid", name=f"h{kh}_{ch}")
                ks0 = kh * KH
                asl = slice(ch * AC2, (ch + 1) * AC2)
                if split:
                    nc.sync.dma_start(ht[:, :KH // 2], hid3[:, ks0:ks0 + KH // 2, asl])
                    nc.sync.dma_start(ht[:, KH // 2:], hid3[:, ks0 + KH // 2:ks0 + KH, asl])
                else:
                    nc.sync.dma_start(ht[:], hid3[:, ks0:ks0 + KH, asl])
                return ht

            # startup: interleave first hid chunk + first W so PE starts ASAP
            h_cur = load_hid(0, 0, split=True)
            load_w(0, 0, split=True)
            nc.sync.dma_start(ones_sb[:], onesd[:, :])
            nc.sync.dma_start(ident[:], identd[:, :])
            nc.sync.dma_start(rmatT_sb[:], rmatT[:, :])
            for m in range(1, 6):
                load_w(0, m)
            for kh in range(2):
                for ch in range(NAC2):
                    asl = slice(ch * AC2, (ch + 1) * AC2)
                    if not (kh == 0 and ch == 0):
                        h_cur = load_hid(kh, ch, split=True)
                    if kh == 0 and ch >= 1:
                        # stagger second K-half weight loads behind hid chunks
                        load_w(1, 2 * (ch - 1))
                        load_w(1, 2 * (ch - 1) + 1)
                    if kh == 1:
                        cos_ch = csp.tile([128, AC2], F32, tag="cos")
                        nc.sync.dma_start(cos_ch[:], cosT[:, asl])
                        sin_ch = csp.tile([128, AC2], F32, tag="sin")
                        nc.sync.dma_start(sin_ch[:], sinT[:, asl])
                    for m in range(6):
                        ps = psA.tile([128, AC2], F32, tag="psA")
                        for k in range(KH):
                            nc.tensor.matmul(
                                ps[:], w_sb[(kh, m)][:, k, :], h_cur[:, k, :],
                                start=(k == 0), stop=(k == KH - 1))
                        if kh == 0:
                            nc.vector.tensor_copy(qk_t[m][:, asl], ps[:])
                            continue
                        nc.vector.tensor_add(qk_t[m][:, asl], qk_t[m][:, asl], ps[:])
                        if m < 5:
                            # fused RoPE: x = x*cos + (rmatT.T@x)*sin
                            rp = psR.tile([128, AC2], F32, tag="psR")
                            nc.tensor.matmul(rp[:], rmatT_sb[:], qk_t[m][:, asl],
                                             start=True, stop=True)
                            tmp = tmp_pool.tile([128, AC2], F32, tag="tmp")
                            nc.vector.tensor_mul(tmp[:], rp[:], sin_ch[:])
                            nc.vector.tensor_mul(qk_t[m][:, asl], qk_t[m][:, asl],
                                                 cos_ch[:])
                            nc.vector.tensor_add(qk_t[m][:, asl], qk_t[m][:, asl],
                                                 tmp[:])
                        else:
                            for jj in range(AC2 // 128):
                                j = ch * (AC2 // 128) + jj
                                pst = psT.tile([128, 128], F32R, tag="psT")
                                nc.tensor.transpose(
                                    pst[:],
                                    qk_t[5][:, j * 128:(j + 1) * 128],
                                    ident[:])
                                nc.vector.tensor_copy(v_sb[:, j, :], pst[:])

        # ---------------- Phase B consts ----------------
        bconsts = ctx.enter_context(tc.tile_pool(name="bconsts", bufs=1))
        mask_sb = bconsts.tile([128, 4, CH], F32)
        nc.sync.dma_start(mask_sb[:], masks.rearrange("i p f -> p i f"))

        wop = ctx.enter_context(tc.tile_pool(name="wo", bufs=1))
        wo_sb = wop.tile([128, HPC, HID // 128, 128], F32R)
        nc.sync.dma_start(
            wo_sb[:], wo.rearrange("(kk p) (mo q) -> p kk mo q", p=128, q=128))

        probs_pool = ctx.enter_context(tc.tile_pool(name="probs", bufs=5))
        attn_pool = ctx.enter_context(tc.tile_pool(name="attnT", bufs=1))
        attn_sb = [attn_pool.tile([128, T], F32R, name=f"attnT{h}") for h in range(HPC)]
        rden_pool = ctx.enter_context(tc.tile_pool(name="rden", bufs=2))
        out_pool = ctx.enter_context(tc.tile_pool(name="outstage", bufs=4))
        psS = ctx.enter_context(tc.tile_pool(name="psS", bufs=2, space="PSUM"))
        psAcc = ctx.enter_context(tc.tile_pool(name="psAcc", bufs=2, space="PSUM"))
        psD = ctx.enter_context(tc.tile_pool(name="psD", bufs=2, space="PSUM"))
        psO = ctx.enter_context(tc.tile_pool(name="psO", bufs=2, space="PSUM"))

        # ---------------- Phase B+C per tq chunk (C deferred one chunk) ---
        def phase_c(c):
            sl = slice(c * CH, (c + 1) * CH)
            for mo in range(HID // 128):
                po = psO.tile([128, CH], F32, tag="po")
                for kk in range(HPC):
                    nc.tensor.matmul(po[:], wo_sb[:, kk, mo, :],
                                     attn_sb[kk][:, sl],
                                     start=(kk == 0), stop=(kk == HPC - 1))
                ob = out_pool.tile([128, CH], F32, tag="ob")
                if mo % 2 == 0:
                    nc.scalar.copy(ob[:], po[:])
                else:
                    nc.vector.tensor_copy(ob[:], po[:])
                nc.sync.dma_start(outT[mo * 128:(mo + 1) * 128, sl], ob[:])

        for c in range(NCH):
            sl = slice(c * CH, (c + 1) * CH)
            nts = (CH // 128) * (c + 1)  # causal: ts tiles 0..nts-1
            for h in range(HPC):
                pa = psAcc.tile([128, CH], F32, tag="acc")
                pd = psD.tile([128, CH], F32, tag="den")
                for j in range(nts):
                    i = j - (CH // 128) * c
                    # diagonal tiles: cols < i*128 are fully masked; shrink to
                    # [off:512] (off capped at 256 to keep f32r N>=256 fast)
                    off = 0 if i < 1 else min(i * 128, 256)
                    csl = slice(c * CH + off, (c + 1) * CH)
                    ss = psS.tile([128, CH], F32, tag="psS")
                    nc.tensor.matmul(ss[:, off:], qk_t[4][:, j * 128:(j + 1) * 128],
                                     qk_t[h][:, csl], start=True, stop=True)
                    pr = probs_pool.tile([128, CH], F32R, tag="probs")
                    nc.scalar.activation(pr[:, off:], ss[:, off:], AF.Exp,
                                         scale=SCALING)
                    if i >= 0:
                        nc.vector.tensor_mul(pr[:, off:], pr[:, off:],
                                             mask_sb[:, i, off:])
                    nc.tensor.matmul(pd[:, off:], ones_sb[:], pr[:, off:],
                                     start=(j == 0), stop=(j == nts - 1))
                    nc.tensor.matmul(pa[:, off:], v_sb[:, j, :], pr[:, off:],
                                     start=(j == 0), stop=(j == nts - 1))
                rden = rden_pool.tile([128, CH], F32, tag="rden")
                nc.vector.reciprocal_approx_fast(out=rden[:], in_=pd[:])
                nc.vector.tensor_mul(attn_sb[h][:, sl], pa[:], rden[:])
            if c >= 1:
                phase_c(c - 1)
        phase_c(NCH - 1)

    nc.compile()
    return nc


def get_nc():
    if "nc" not in _CACHE:
        _CACHE["nc"] = _build_nc()
    return _CACHE["nc"]


def prep_in_maps(hidden_states, cos, sin, Wq, Wk, Wv, Wo):
    hidT = np.ascontiguousarray(hidden_states.T)
    cosT = np.ascontiguousarray(cos.T)
    sinT = np.ascontiguousarray(sin.T)
    # signed rotate-half permutation (as lhsT): rot = rmatT.T @ x
    rmatT = np.zeros((D, D), dtype=np.float32)
    half = D // 2
    rmatT[np.arange(half) + half, np.arange(half)] = -1.0  # rot[d<64] = -x[d+64]
    rmatT[np.arange(half), np.arange(half) + half] = 1.0   # rot[d>=64] = x[d-64]
    # causal diag masks: mask[i][p][f] = 1 if i*128+p <= f
    i_idx = np.arange(4)[:, None, None] * 128
    p_idx = np.arange(128)[None, :, None]
    f_idx = np.arange(CH)[None, None, :]
    masks = ((i_idx + p_idx) <= f_idx).astype(np.float32)

    in_maps = []
    for c in range(NCORES):
        wqkv = np.concatenate([
            Wq[:, c * QW:(c + 1) * QW],
            Wk[:, c * D:(c + 1) * D],
            Wv[:, c * D:(c + 1) * D],
        ], axis=1)
        in_maps.append({
            "hidT": hidT,
            "wqkv": np.ascontiguousarray(wqkv),
            "wo": np.ascontiguousarray(Wo[c * QW:(c + 1) * QW, :]),
            "cosT": cosT,
            "sinT": sinT,
            "rmatT": rmatT,
            "onesd": np.ones((128, 128), dtype=np.float32),
            "identd": np.eye(128, dtype=np.float32),
            "masks": masks,
        })
    return in_maps


def postprocess(results):
    acc = results[0]["outT_p"].copy()
    for r in results[1:]:
        acc += r["outT_p"]
    return np.ascontiguousarray(acc.T).astype(np.float32)


def kernel(hidden_states, position_ids, cos, sin, Wq, Wk, Wv, Wo):
    from concourse.bass_utils import run_bass_kernel_spmd
    nc = get_nc()
    in_maps = prep_in_maps(hidden_states, cos, sin, Wq, Wk, Wv, Wo)
    res = run_bass_kernel_spmd(nc, in_maps, core_ids=list(range(NCORES)))
    return postprocess(res.results)
